# revision 31
# baseline (speedup 1.0000x reference)
"""Trainium2 Bass kernel for batched 9x9-token MHSA with decomposed relative
position bias (1x1-conv QKV projection).

Strategy: pure data parallel over batch (B=1024 -> 128 per core x 8 cores).
Per core (all GEMMs bf16 with fp32 PSUM accumulate; rel_err ~8.6e-3 vs the
2e-2 gate):
  - QK projection GEMM channel-major (out [o, (b,n)]), moving dim 324.
    Relative-position table R = rel_h+rel_w (+ k bias) is folded into K
    during the PSUM->SBUF epilogue, so scores = Q.(K+R) in one matmul.
  - V projection GEMM token-major per batch (out [n, dv]), moving dim 512.
  - Scores computed transposed: S^T[m,n] = sum_d k'[d,m] q[d,n] via
    matmul(lhsT=k', rhs=q), head parities packed into PE row-halves via
    tile_position. Softmax runs over partitions (m): no max subtraction
    (logits bounded by ~33, exp<=1.4e14, safe in fp32); denominator
    obtained by appending a ones-row to V so the AV matmul emits
    unnormalized output rows 0..63 and the denominator in row 64.
  - exp on ScalarE (fp32 PSUM -> bf16 SBUF; ACT kept exp-only — any other
    ACT work delays psS PSUM bank release and stalls the next S-run),
    AV matmul bf16, PSUM->SBUF output copies on VectorE, bf16 output
    (host divides by the denominator during unshard).
  - x / weights / rel-table host-packed so every stream is ONE DMA per
    chunk (DIRECT2D issue costs ~750ns of SP sequencer each); small
    consts issue on the ACT DGE so they don't delay the first x chunk.

Measured (NTFF HW profile, per-core): 409us vs 558us for the fp32r
baseline; PE matmul stream ~387us busy with <2us of gaps.

Self-contained: hardcodes B=1024, DM=512, H=8, D=64, N=81, 8 cores.
"""

import os
import sys

import ml_dtypes
import numpy as np

for _p in ("/opt/trn_rl_repo", "/root/.axon_site/_ro/trn_rl_repo"):
    if os.path.isdir(_p) and _p not in sys.path:
        sys.path.insert(0, _p)

import concourse.bass as bass
import concourse.tile as tile
from concourse import bacc
from concourse import mybir
from concourse.alu_op_type import AluOpType
from concourse.bass_utils import run_bass_kernel_spmd

F32 = mybir.dt.float32
F32R = mybir.dt.float32r
BF16 = mybir.dt.bfloat16
AF = mybir.ActivationFunctionType

B, DM, H, D, N = 1024, 512, 8, 64, 81
NCORES = 8
B_CORE = B // NCORES  # 128
NB = 4                # batches per chunk
NCOLS = NB * N        # 324 GEMM moving columns per chunk


def build_kernel(n_b=B_CORE, reps=1, qkv_bf16=False, scores_bf16=False):
    assert n_b % NB == 0
    nchunks = n_b // NB
    gdt = BF16 if qkv_bf16 else F32R   # projection-GEMM operand dtype
    sdt = BF16 if scores_bf16 else F32  # q/k SBUF tile dtype (scores matmul)

    nc = bacc.Bacc()
    # x pre-transposed on host to [128, chunk, kc, b, n] so each chunk loads
    # with ONE DMA of 128 contiguous per-partition runs (DIRECT2D issue on the
    # SP sequencer costs ~650ns each — 4 issues/chunk was 2.6us/chunk).
    xd = nc.dram_tensor("x", [128, nchunks * 4 * NCOLS], gdt,
                        kind="ExternalInput")
    # W^T packed [128, kc, o] — single-DMA constant load; same for rp/bq.
    wtd = nc.dram_tensor("wt", [128, 4 * 3 * DM], gdt, kind="ExternalInput")
    bqd = nc.dram_tensor("bq", [128, 4], F32, kind="ExternalInput")       # q bias
    rpd = nc.dram_tensor("rp", [128, 4 * N], F32, kind="ExternalInput")   # rel+bk
    bvd = nc.dram_tensor("bv", [1, DM], F32, kind="ExternalInput")        # v bias row
    # out in device-native layout [pair][d+denom][b par hh n]; row D holds the
    # softmax denominator — the final normalize division happens on the host
    # during unsharding. One fully-contiguous store per batch pair. bf16:
    # halves the DVE PSUM->SBUF copy write cost and the store bytes.
    outd = nc.dram_tensor(
        "out", [n_b // 2, D + 1, 2 * 2 * 4 * N], BF16, kind="ExternalOutput"
    )

    with tile.TileContext(nc) as tc:
        with (
            tc.tile_pool(name="const", bufs=1) as cpool,
            tc.tile_pool(name="xin", bufs=3) as xpool,
            tc.tile_pool(name="qk", bufs=2) as qkpool,
            tc.tile_pool(name="vaug", bufs=3) as vpool,
            tc.tile_pool(name="emat", bufs=4) as epool,
            tc.tile_pool(name="small", bufs=4) as spool,
            tc.tile_pool(name="outs", bufs=3) as opool,
            tc.tile_pool(name="ps_qk", bufs=2, space="PSUM") as ps_qk,
            tc.tile_pool(name="ps_v", bufs=2, space="PSUM") as ps_v,
            tc.tile_pool(name="ps_s", bufs=2, space="PSUM") as ps_s,
            tc.tile_pool(name="ps_av", bufs=2, space="PSUM") as ps_av,
        ):
            # ---- constants (loaded once, one DMA each) ----
            # wt + x(chunk 0) issue on the SP DGE; the small consts issue on
            # the ACT DGE so their ~750ns issues don't delay x chunk 0.
            wt4 = cpool.tile([128, 4, 3 * DM], gdt, tag="wt")
            nc.sync.dma_start(out=wt4[:].rearrange("p k o -> p (k o)"),
                              in_=wtd[:])
            rp4 = cpool.tile([128, 4, N], F32, tag="rp")
            nc.scalar.dma_start(out=rp4[:].rearrange("p m n -> p (m n)"),
                                in_=rpd[:])
            bq4 = cpool.tile([128, 4, 1], F32, tag="bq")
            nc.scalar.dma_start(out=bq4[:].rearrange("p m o -> p (m o)"),
                                in_=bqd[:])
            bvb = cpool.tile([N, DM], F32, tag="bvb")
            nc.scalar.dma_start(out=bvb[:], in_=bvd[0:1, :].to_broadcast([N, DM]))

            state = {}  # carries one chunk's tiles to the next iteration

            def gemm(c):
                b0 = c * NB
                xt = xpool.tile([128, 4, NB, N], gdt, tag="x")
                nc.sync.dma_start(
                    out=xt[:].rearrange("p k b n -> p (k b n)"),
                    in_=xd[:, c * 4 * NCOLS:(c + 1) * 4 * NCOLS],
                )

                # q,k channel-major GEMM: out[o, (b,n)] for o in 0..1024
                q_sb, k_sb = [], []
                for mo in range(8):
                    ps = ps_qk.tile([128, NCOLS], F32, tag="psqk")
                    for kc in range(4):
                        nc.tensor.matmul(
                            ps[:],
                            lhsT=wt4[:, kc, mo * 128:(mo + 1) * 128],
                            rhs=xt[:, kc].rearrange("p b n -> p (b n)"),
                            start=(kc == 0),
                            stop=(kc == 3),
                        )
                    if mo < 4:  # q: add bias on ScalarE while copying out
                        t = qkpool.tile([128, NCOLS], sdt, tag=f"q{mo}")
                        nc.scalar.activation(t[:], ps[:], AF.Identity,
                                             bias=bq4[:, mo])
                        q_sb.append(t)
                    else:  # k: add (rel bias + k bias), broadcast over batch
                        t = qkpool.tile([128, NCOLS], sdt, tag=f"k{mo - 4}")
                        nc.vector.tensor_tensor(
                            t[:].rearrange("p (b n) -> p b n", b=NB),
                            ps[:].rearrange("p (b n) -> p b n", b=NB),
                            rp4[:, mo - 4].unsqueeze(1).broadcast_to(
                                [128, NB, N]
                            ),
                            AluOpType.add,
                        )
                        k_sb.append(t)

                # v token-major GEMM per batch + bias + ones column (bf16)
                v_aug = []
                for j in range(NB):
                    ps = ps_v.tile([N, DM], F32, tag="psv")
                    for kc in range(4):
                        nc.tensor.matmul(
                            ps[:],
                            lhsT=xt[:, kc, j, :],
                            rhs=wt4[:, kc, 2 * DM:3 * DM],
                            start=(kc == 0),
                            stop=(kc == 3),
                        )
                    va = vpool.tile([N, H * (D + 1)], BF16, tag="vaug")
                    nc.vector.tensor_tensor(
                        va[:].rearrange("p (h e) -> p h e", h=H)[:, :, 0:D],
                        ps[:].rearrange("p (h d) -> p h d", h=H),
                        bvb[:].rearrange("p (h d) -> p h d", h=H),
                        AluOpType.add,
                    )
                    nc.vector.memset(
                        va[:].rearrange("p (h e) -> p h e", h=H)[:, :, D:D + 1], 1.0
                    )
                    v_aug.append(va)
                return {"q": q_sb, "k": k_sb, "v": v_aug, "b0": b0}

            def attention(st):
                q_sb, k_sb, v_aug, b0 = st["q"], st["k"], st["v"], st["b0"]
                ot = None
                for j in range(NB):
                    b = b0 + j
                    js = slice(j * N, (j + 1) * N)
                    if j % 2 == 0:  # one output tile per batch pair
                        ot = opool.tile([D + 1, 2 * 2 * 4 * N], BF16, tag="ot")
                    # scores transposed: S^T = k'.T-contracted over d.
                    # Grouped by head parity: tile `par` holds heads 2*hh+par,
                    # so every matmul into one PSUM tile has the same lhsT
                    # base partition (mixing 0/64 in one fp32 group breaks HW).
                    # parities interleaved: consecutive matmuls use disjoint
                    # PE row strips (0-63 vs 64-127) and different PSUM banks,
                    # so the PE can overlap them
                    psS = [
                        ps_s.tile([N, 4 * N], F32, tag="pss", name=f"pss{j}_{p}")
                        for p in range(2)
                    ]
                    for hh in range(4):
                        for par in range(2):
                            po = par * 64
                            nc.tensor.matmul(
                                psS[par][:, hh * N:(hh + 1) * N],
                                lhsT=k_sb[hh][po:po + 64, js],
                                rhs=q_sb[hh][po:po + 64, js],
                                start=True,
                                stop=True,
                                tile_position=(po, 0),
                            )
                    emat = []
                    for par in range(2):
                        e = epool.tile([N, 4 * N], BF16, tag="e")
                        nc.scalar.activation(e[:], psS[par][:], AF.Exp)
                        emat.append(e)
                    # AV with ones-row: rows 0..63 unnormalized out, row 64 denom
                    psA = []
                    for par in range(2):
                        ps = ps_av.tile([D + 1, 4 * N], F32, tag="psav")
                        for hh in range(4):
                            h = 2 * hh + par
                            nc.tensor.matmul(
                                ps[:, hh * N:(hh + 1) * N],
                                lhsT=v_aug[j][:, h * (D + 1):(h + 1) * (D + 1)],
                                rhs=emat[par][:, hh * N:(hh + 1) * N],
                                start=True,
                                stop=True,
                            )
                        psA.append(ps)
                    # ot free layout is (b01, par, hh, n); channel h = 2*hh+par
                    # both PSUM->SBUF copies on DVE: an ACT copy would get
                    # scheduled between exp halves and delay psS bank release
                    joff = (j % 2) * 2 * 4 * N
                    nc.vector.tensor_copy(
                        ot[:, joff:joff + 4 * N], psA[0][:]
                    )
                    nc.vector.tensor_copy(
                        ot[:, joff + 4 * N:joff + 2 * 4 * N], psA[1][:]
                    )
                    if j % 2 == 1:
                        nc.sync.dma_start(out=outd[b // 2], in_=ot[:])

            # software pipeline: attention for chunk c-1 is emitted before
            # GEMM for chunk c so PE never stalls on ACT/DVE epilogues.
            # reps>1 repeats the body (same data) for slope-based HW timing.
            chunk_ids = list(range(nchunks)) * reps
            for c in range(len(chunk_ids) + 1):
                if c > 0:
                    attention(state)
                if c < len(chunk_ids):
                    state = gemm(chunk_ids[c])

    if not nc.is_finalized():
        nc.finalize()
    return nc


_CACHE = {}

# proven-correct fastest config used by kernel(); flipped as variants validate
QKV_BF16 = True
SCORES_BF16 = True


def _get_nc(n_b, reps=1, qkv_bf16=QKV_BF16, scores_bf16=SCORES_BF16):
    key = (n_b, reps, qkv_bf16, scores_bf16)
    if key not in _CACHE:
        _CACHE[key] = build_kernel(n_b, reps, qkv_bf16, scores_bf16)
    return _CACHE[key]


def _prep_inputs(x, qkv_w, qkv_b, rel_h, rel_w, qkv_bf16=QKV_BF16):
    gnp = ml_dtypes.bfloat16 if qkv_bf16 else np.float32
    nchunks = B_CORE // NB
    # per-core x packed [128, chunk, kc, b, n] (one DMA per chunk)
    x = np.asarray(x, dtype=np.float32).reshape(
        NCORES, nchunks, NB, 4, 128, N
    )
    x = np.ascontiguousarray(x.transpose(0, 4, 1, 3, 2, 5)).reshape(
        NCORES, 128, nchunks * 4 * NB * N
    ).astype(gnp)
    qkv_w = np.asarray(qkv_w, dtype=np.float32)
    qkv_b = np.asarray(qkv_b, dtype=np.float32)
    wt = np.ascontiguousarray(                                  # [128, kc*1536]
        qkv_w.T.reshape(4, 128, 3 * DM).transpose(1, 0, 2)
    ).reshape(128, 4 * 3 * DM).astype(gnp)
    bq = np.ascontiguousarray(qkv_b[0:DM].reshape(4, 128).T)    # [128, 4]
    rel = (np.asarray(rel_h, np.float32) + np.asarray(rel_w, np.float32))
    rp = rel.reshape(DM, N) + qkv_b[DM:2 * DM].reshape(DM, 1)
    rp = np.ascontiguousarray(
        rp.reshape(4, 128, N).transpose(1, 0, 2)
    ).reshape(128, 4 * N)                                       # [128, 4*81]
    bv = np.ascontiguousarray(qkv_b[2 * DM:3 * DM].reshape(1, DM))
    return x, wt, bq, rp, bv


def kernel(x, qkv_w, qkv_b, rel_h, rel_w, _trace=False):
    xs, wt, bq, rp, bv = _prep_inputs(x, qkv_w, qkv_b, rel_h, rel_w)
    nc = _get_nc(B_CORE)
    in_maps = [
        {"x": xs[i], "wt": wt, "bq": bq, "rp": rp, "bv": bv}
        for i in range(NCORES)
    ]
    res = run_bass_kernel_spmd(
        nc, in_maps, core_ids=list(range(NCORES)), trace=_trace
    )
    # decode device layout [pair, d|denom, b01, par, hh, n] -> [B, DM, N];
    # row D is the softmax denominator (normalize here during unshard)
    out = np.stack(
        [r["out"].astype(np.float32) for r in res.results], axis=0
    )
    out = out.reshape(NCORES, B_CORE // 2, D + 1, 2, 2, 4, N)
    out = out[:, :, 0:D] / out[:, :, D:D + 1]
    out = out.transpose(0, 1, 3, 5, 4, 2, 6)  # core, pair, b01, hh, par, d, n
    out = out.reshape(B, DM, N)
    if _trace:
        kernel.last_results = res
    return np.ascontiguousarray(out.reshape(B, DM, 9, 9))

